# revision 19
# baseline (speedup 1.0000x reference)
"""Trainium2 Bass kernel for nn_LstmConv (GNN message passing + LSTMCell).

Sharding: dst nodes load-balanced across 8 cores (permuted into 49 tiles of
128 slots per core). Per core, edge features are fetched with few BIG
InstDMAGatherAnt gathers (bf16 rows, int16 indices against a half-split
feature table, ~2 gathers per 2-tile group) to amortize the ~1us SWDGE
fixed cost; the segment sum is a chain of bf16 one-hot matmuls into PSUM,
and the LSTMCell runs as two PE matmuls + ACT/DVE epilogue per tile.
Output is written transposed and reassembled on the host.
"""

import sys, os

sys.path.insert(0, "/opt/trn_rl_repo")
sys.path.insert(0, os.path.dirname(os.path.abspath(__file__)))

import numpy as np
from ml_dtypes import bfloat16

N_NODES = 50000
N_EDGES = 800000
H = 128
MSG = 64
P = 128
NCORES = 8
TILES = 49
SLOTS = TILES * P          # 6272 per core
HALF = 25000               # feature table split point (int16 index range)
GT = 2                     # tiles per gather group
MAXCH = 8                  # max 128-idx chunks per dma_gather (SWDGE ring cap)

LAST_EXEC_NS = None
TRACE = False


def _wrap16(idx, width):
    """Pack linear index list into [128, width] (idx j at [j%16, j//16],
    replicated across the 8 gpsimd cores)."""
    out = np.zeros((P, width), np.uint16)
    n = len(idx)
    cols = (n + 15) // 16
    blk = np.zeros((16, width), np.uint16)
    flat = np.zeros(cols * 16, np.uint16)
    flat[:n] = idx
    blk[:, :cols] = flat.reshape(cols, 16).T
    out[:] = np.tile(blk, (8, 1))
    return out


def _host_prep(feat, src0, dst0, src1, dst1, W_ih, W_hh, b_ih, b_hh):
    deg0 = np.bincount(dst0, minlength=N_NODES)
    deg1 = np.bincount(dst1, minlength=N_NODES)
    w = deg0 + deg1

    # snake-assign nodes (sorted by degree desc) into 392 tiles of <=128
    n_tiles_g = NCORES * TILES
    order = np.argsort(-w, kind="stable")
    tile_of_node = np.empty(N_NODES, np.int32)
    pos_in_tile = np.empty(N_NODES, np.int32)
    tcnt = np.zeros(n_tiles_g, np.int32)
    idx = 0
    fwd = True
    while idx < N_NODES:
        rng = range(n_tiles_g) if fwd else range(n_tiles_g - 1, -1, -1)
        for t in rng:
            if idx >= N_NODES:
                break
            if tcnt[t] < P:
                tile_of_node[order[idx]] = t
                pos_in_tile[order[idx]] = tcnt[t]
                tcnt[t] += 1
                idx += 1
        fwd = not fwd

    # balance tiles over cores by weight: snake over tiles sorted by weight
    tile_w = np.zeros(n_tiles_g, np.int64)
    np.add.at(tile_w, tile_of_node, w)
    torder = np.argsort(-tile_w, kind="stable")
    core_of_tile = np.empty(n_tiles_g, np.int32)
    tl_of_tile = np.empty(n_tiles_g, np.int32)
    k = 0
    fwd = True
    for rnd in range(TILES):
        cr = range(NCORES) if fwd else range(NCORES - 1, -1, -1)
        for c in cr:
            core_of_tile[torder[k]] = c
            tl_of_tile[torder[k]] = rnd
            k += 1
        fwd = not fwd

    core_of_node = core_of_tile[tile_of_node]
    slot_of_node = tl_of_tile[tile_of_node] * P + pos_in_tile  # slot within core

    # node_of_slot per core (-1 = ghost)
    node_of_slot = -np.ones((NCORES, SLOTS), np.int64)
    node_of_slot[core_of_node, slot_of_node] = np.arange(N_NODES)

    # per-node combined scales a_e = 1/max(cnt_e,1) * 1/max(has0+has1,1)
    has0 = (deg0 > 0).astype(np.float32)
    has1 = (deg1 > 0).astype(np.float32)
    invc = 1.0 / np.maximum(has0 + has1, 1.0)
    a0 = invc / np.maximum(deg0, 1.0)
    a1 = invc / np.maximum(deg1, 1.0)

    # per-core (tile, etype, half) edge sub-groups, sorted by slot
    groups = {}   # (core, tl, e, half) -> (srcs_rel, slot_pos)
    for e, (src, dst) in enumerate(((src0, dst0), (src1, dst1))):
        c = core_of_node[dst]
        s = slot_of_node[dst]
        hb = (src >= HALF).astype(np.int64)
        key = (((c * TILES + (s // P)) * 2 + hb) * P + (s % P)).astype(np.int64)
        o = np.argsort(key, kind="stable")
        src_s, c_s, s_s, hb_s = src[o], c[o], s[o], hb[o]
        gkey = (c_s * TILES + s_s // P) * 2 + hb_s
        bounds = np.searchsorted(gkey, np.arange(NCORES * TILES * 2 + 1))
        for g in range(NCORES * TILES * 2):
            lo, hi = bounds[g], bounds[g + 1]
            cc, rem = divmod(g, TILES * 2)
            tl, hh = divmod(rem, 2)
            groups[(cc, tl, e, hh)] = (
                src_s[lo:hi] - HALF * hh, (s_s[lo:hi] % P).astype(np.float64))

    # common chunk counts per (tl, e, half): max over cores
    K_teh = np.zeros((TILES, 2, 2), np.int32)
    for tl in range(TILES):
        for e in range(2):
            for hh in range(2):
                m = max(len(groups[(c, tl, e, hh)][0]) for c in range(NCORES))
                K_teh[tl, e, hh] = (m + 127) // 128

    # chunk layout: per gather group g of GT tiles:
    #   [tiles' (e0,h0) (e1,h0) chunks | tiles' (e0,h1) (e1,h1) chunks]
    # meta is identical across cores (shapes/counts only).
    n_groups = (TILES + GT - 1) // GT
    chunk_meta = []        # (tl, e, half) per global chunk position
    tile_cols = [[[], []] for _ in range(TILES)]
    gathers = []           # (half, chunk_lo, nch) per gather, in group order
    group_spans = []       # (chunk_lo, nch_total) per group (for hot/is_equal)
    for g in range(n_groups):
        tls = range(g * GT, min((g + 1) * GT, TILES))
        g_lo = len(chunk_meta)
        for hh in range(2):
            h_lo = len(chunk_meta)
            for tl in tls:
                for e in range(2):
                    for _ in range(K_teh[tl, e, hh]):
                        tile_cols[tl][e].append(len(chunk_meta))
                        chunk_meta.append((tl, e, hh))
            # split the half-span into gathers of <= MAXCH chunks
            nch_h = len(chunk_meta) - h_lo
            s = h_lo
            while nch_h > 0:
                take = min(nch_h, MAXCH)
                gathers.append((hh, s, take))
                s += take
                nch_h -= take
        group_spans.append((g_lo, len(chunk_meta) - g_lo))
    CT2 = len(chunk_meta)

    # per-core data arrays in chunk order
    per_core = []
    for c in range(NCORES):
        idx_flat = np.zeros(CT2 * P, np.uint16)
        doff = np.full((P, CT2), 255.0, np.float32)
        pos = {}
        for tl in range(TILES):
            for e in range(2):
                for hh in range(2):
                    pos[(tl, e, hh)] = 0
        # fill per (tl,e,hh) spans in chunk order
        ch = 0
        while ch < CT2:
            tl, e, hh = chunk_meta[ch]
            K = K_teh[tl, e, hh]
            srcs, pps = groups[(c, tl, e, hh)]
            n = len(srcs)
            a = np.zeros(K * P, np.uint16)
            a[:n] = srcs
            d = np.full(K * P, 255.0, np.float32)
            d[:n] = pps
            idx_flat[ch * P : (ch + K) * P] = a
            doff[:, ch : ch + K] = d.reshape(K, P).T
            ch += K
        # wrap16 index layout per gather span
        gw_parts = []
        for hh, lo, nch in gathers:
            if nch == 0:
                continue
            gw_parts.append(
                _wrap16(idx_flat[lo * P : (lo + nch) * P], nch * 8))
        gidx16 = np.concatenate(gw_parts, axis=1).view(np.int16).copy()

        # scales: sc[p, tl*2+e] = a_e[slot tl*128+p]
        sc = np.zeros((P, TILES * 2), np.float32)
        av0 = np.where(node_of_slot[c] >= 0, a0[np.maximum(node_of_slot[c], 0)], 0.0)
        av1 = np.where(node_of_slot[c] >= 0, a1[np.maximum(node_of_slot[c], 0)], 0.0)
        for tl in range(TILES):
            sc[:, tl * 2] = av0[tl * P : (tl + 1) * P]
            sc[:, tl * 2 + 1] = av1[tl * P : (tl + 1) * P]
        # local node features, transposed, bf16
        sl = node_of_slot[c]
        floc = np.zeros((SLOTS, H), np.float32)
        floc[sl >= 0] = feat[sl[sl >= 0]]
        per_core.append(dict(gidx16=gidx16, doff=doff.astype(bfloat16), scales=sc,
                             featloc=floc.T.astype(bfloat16).copy()))

    # half-split feature tables, bf16
    featA = feat[:HALF].astype(bfloat16)
    featB = np.zeros((HALF, H), np.float32)
    featB[: N_NODES - HALF] = feat[HALF:]
    featB = featB.astype(bfloat16)

    # gate order [i, f, g, o] (PyTorch native)
    wih = W_ih.T.astype(bfloat16).copy()              # [128, 256]
    whh = W_hh.T.astype(bfloat16).copy()              # [64, 256]
    bt = (b_ih + b_hh).astype(np.float32)
    biasT = np.stack([bt[:128], bt[128:]], axis=1).copy()  # [128, 2]
    iota = np.tile(np.arange(P, dtype=np.float32)[None, :], (P, 1)).astype(bfloat16)

    shared = dict(featA=featA, featB=featB, wih=wih, whh=whh, biasT=biasT,
                  iota=iota)
    meta = dict(CT2=CT2, gathers=gathers, group_spans=group_spans,
                tile_cols=tile_cols, n_groups=n_groups,
                GW=sum(nch * 8 for _, _, nch in gathers))
    return per_core, shared, node_of_slot, meta


_WS = [0]


def _split_multi_waits(nc, mybir, max_waits=1):
    """This container's walrus rejects >1 sync wait per instruction; split
    extra waits onto single-wait NoOps placed just before the instruction."""
    for fn in nc.m.functions:
        for bb in fn.blocks:
            new = []
            for ins in bb.instructions:
                si = ins.sync_info
                if si is not None and len(si.on_wait) > max_waits:
                    waits = list(si.on_wait)
                    for w in waits[:-max_waits]:
                        _WS[0] += 1
                        nop = mybir.InstNoOp(
                            name=f"I-waitsplit-{_WS[0]}", ins=[], outs=[]
                        )
                        nop.engine = ins.engine
                        nop.sync_info = mybir.SyncInfo(on_wait=[w], on_update=[])
                        new.append(nop)
                    si.on_wait = waits[-max_waits:]
                new.append(ins)
            bb.instructions[:] = new


def _build_nc(meta):
    from concourse import bass, mybir, tile, library_config
    from concourse.masks import make_identity

    f32, bf16, i16 = mybir.dt.float32, mybir.dt.bfloat16, mybir.dt.int16
    CT2 = meta["CT2"]
    gathers = meta["gathers"]
    group_spans = meta["group_spans"]
    tile_cols = meta["tile_cols"]
    n_groups = meta["n_groups"]
    GW = meta["GW"]

    nc = bass.Bass(num_swdge_queues=4)
    featA_d = nc.declare_dram_parameter("featA", [HALF, H], bf16, isOutput=False)
    featB_d = nc.declare_dram_parameter("featB", [HALF, H], bf16, isOutput=False)
    gidx_d = nc.declare_dram_parameter("gidx16", [P, GW], i16, isOutput=False)
    doff_d = nc.declare_dram_parameter("doff", [P, CT2], bf16, isOutput=False)
    sc_d = nc.declare_dram_parameter("scales", [P, TILES * 2], f32, isOutput=False)
    wih_d = nc.declare_dram_parameter("wih", [P, 256], bf16, isOutput=False)
    whh_d = nc.declare_dram_parameter("whh", [64, 256], bf16, isOutput=False)
    bias_d = nc.declare_dram_parameter("biasT", [P, 2], f32, isOutput=False)
    iota_d = nc.declare_dram_parameter("iota", [P, P], bf16, isOutput=False)
    floc_d = nc.declare_dram_parameter("featloc", [P, SLOTS], bf16, isOutput=False)
    outT_d = nc.declare_dram_parameter("outT", [P, SLOTS], f32, isOutput=True)

    KBUF = max(n for _, n in group_spans)

    with tile.TileContext(nc) as tc:
        with (
            tc.tile_pool(name="const", bufs=1) as cp,
            tc.tile_pool(name="gb", bufs=3) as gbp,
            tc.tile_pool(name="hot", bufs=2) as hp,
            tc.tile_pool(name="ep", bufs=2) as ep,
            tc.tile_pool(name="psm", bufs=2, space="PSUM") as psm,
            tc.tile_pool(name="pst", bufs=1, space="PSUM") as pst,
            tc.tile_pool(name="psg", bufs=1, space="PSUM") as psgp,
        ):
            nc.gpsimd.load_library(library_config.mlp)
            niregs = {}
            for hh, lo, nch in gathers:
                if nch and nch * P not in niregs:
                    niregs[nch * P] = nc.gpsimd.to_reg(nch * P)
            gidx = cp.tile([P, GW], i16)
            for q in range(8):
                lo, hi = GW * q // 8, GW * (q + 1) // 8
                nc.sync.dma_start(out=gidx[:, lo:hi], in_=gidx_d[:, lo:hi])
            doff = cp.tile([P, CT2], bf16)
            for q in range(4):
                lo, hi = CT2 * q // 4, CT2 * (q + 1) // 4
                nc.sync.dma_start(out=doff[:, lo:hi], in_=doff_d[:, lo:hi])
            sc = cp.tile([P, TILES * 2], f32)
            nc.sync.dma_start(out=sc[:], in_=sc_d[:])
            wih = cp.tile([P, 256], bf16)
            nc.sync.dma_start(out=wih[:], in_=wih_d[:])
            whh = cp.tile([64, 256], bf16)
            nc.sync.dma_start(out=whh[:], in_=whh_d[:])
            bias = cp.tile([P, 2], f32)
            nc.sync.dma_start(out=bias[:], in_=bias_d[:])
            iota = cp.tile([P, P], bf16)
            nc.sync.dma_start(out=iota[:], in_=iota_d[:])
            featloc = cp.tile([P, SLOTS], bf16)
            nc.sync.dma_start(out=featloc[:], in_=floc_d[:])
            iota_ident = cp.tile([P, P], f32)
            make_identity(nc, iota_ident[:])

            gi = 0      # gather index
            icol = 0    # idx16 column cursor
            for g in range(n_groups):
                g_lo, g_nch = group_spans[g]
                gb = gbp.tile([P, KBUF, P], bf16, tag="gb")
                while gi < len(gathers) and gathers[gi][1] < g_lo + g_nch:
                    hh, lo, nch = gathers[gi]
                    gi += 1
                    if nch == 0:
                        continue
                    nc.gpsimd.dma_gather(
                        out_ap=gb[:, lo - g_lo : lo - g_lo + nch, :],
                        in_ap=(featA_d if hh == 0 else featB_d)[:],
                        idxs_ap=gidx[:, icol : icol + nch * 8],
                        num_idxs=nch * P,
                        num_idxs_reg=niregs[nch * P],
                        elem_size=H,
                        queue_num=gi % 4,
                    )
                    icol += nch * 8
                hot = hp.tile([P, KBUF * P], bf16, tag="hot")
                nc.vector.tensor_tensor(
                    out=hot[:, : g_nch * P],
                    in0=doff[:, g_lo : g_lo + g_nch].to_broadcast([P, g_nch, P]),
                    in1=iota[:, None, :].to_broadcast([P, g_nch, P]),
                    op=mybir.AluOpType.is_equal,
                )
                for tl in range(g * GT, min((g + 1) * GT, TILES)):
                    pms = []
                    for e in range(2):
                        cols = tile_cols[tl][e]
                        pm = psm.tile([P, P], f32, tag=f"m{e}")
                        for j, ch in enumerate(cols):
                            k = ch - g_lo
                            nc.tensor.matmul(
                                out=pm[:],
                                lhsT=hot[:, k * P : (k + 1) * P],
                                rhs=gb[:, k, :],
                                start=(j == 0), stop=(j == len(cols) - 1),
                            )
                        pms.append(pm)
                    # rst in [node, h]: per-partition scale, then transpose
                    rnh = ep.tile([P, P], f32, tag="rnh")
                    t1 = ep.tile([P, P], f32, tag="t1")
                    nc.vector.tensor_scalar(
                        out=rnh[:], in0=pms[0][:],
                        scalar1=sc[:, tl * 2 : tl * 2 + 1], scalar2=None,
                        op0=mybir.AluOpType.mult,
                    )
                    nc.vector.tensor_scalar(
                        out=t1[:], in0=pms[1][:],
                        scalar1=sc[:, tl * 2 + 1 : tl * 2 + 2], scalar2=None,
                        op0=mybir.AluOpType.mult,
                    )
                    nc.vector.tensor_tensor(
                        out=rnh[:], in0=rnh[:], in1=t1[:], op=mybir.AluOpType.add
                    )
                    ptb = pst.tile([P, P], f32, tag="pt")
                    nc.tensor.transpose(out=ptb[:], in_=rnh[:], identity=iota_ident[:])
                    rstf = ep.tile([P, P], f32, tag="rstf")
                    rstb = ep.tile([P, P], bf16, tag="rstb")
                    nc.scalar.activation(
                        out=rstf[:], in_=ptb[:],
                        func=mybir.ActivationFunctionType.Copy,
                    )
                    nc.vector.tensor_copy(out=rstb[:], in_=ptb[:])
                    # gates
                    pg = []
                    for half in range(2):
                        g_ps = psgp.tile([P, P], f32, tag=f"pg{half}")
                        nc.tensor.matmul(
                            out=g_ps[:], lhsT=wih[:, half * P : (half + 1) * P],
                            rhs=featloc[:, tl * P : (tl + 1) * P],
                            start=True, stop=False,
                        )
                        nc.tensor.matmul(
                            out=g_ps[:], lhsT=whh[:, half * P : (half + 1) * P],
                            rhs=rstb[0:64, :], start=False, stop=True,
                        )
                        pg.append(g_ps)
                    # gates halves: pg0 = [i; f], pg1 = [g; o]
                    sif = ep.tile([P, P], f32, tag="sif")
                    nc.scalar.activation(
                        out=sif[:], in_=pg[0][:],
                        func=mybir.ActivationFunctionType.Sigmoid, bias=bias[:, 0:1],
                    )
                    sog = ep.tile([P, P], f32, tag="sog")
                    nc.scalar.activation(
                        out=sog[0:64, :], in_=pg[1][0:64, :],
                        func=mybir.ActivationFunctionType.Tanh, bias=bias[0:64, 1:2],
                    )
                    nc.scalar.activation(
                        out=sog[64:128, :], in_=pg[1][64:128, :],
                        func=mybir.ActivationFunctionType.Sigmoid, bias=bias[64:128, 1:2],
                    )
                    outsb = ep.tile([P, P], f32, tag="outsb")
                    t2 = ep.tile([64, P], f32, tag="t2")
                    tt = ep.tile([P, P], f32, tag="tt")
                    nc.vector.tensor_tensor(
                        out=t2[:], in0=sif[0:64, :], in1=sog[0:64, :],
                        op=mybir.AluOpType.mult,
                    )
                    nc.scalar.activation(
                        out=tt[64:128, :], in_=t2[:],
                        func=mybir.ActivationFunctionType.Copy,
                    )
                    nc.vector.tensor_tensor(
                        out=outsb[64:128, :], in0=sif[64:128, :], in1=rstf[64:128, :],
                        op=mybir.AluOpType.mult,
                    )
                    nc.vector.tensor_tensor(
                        out=outsb[64:128, :], in0=outsb[64:128, :], in1=tt[64:128, :],
                        op=mybir.AluOpType.add,
                    )
                    nc.scalar.activation(
                        out=tt[64:128, :], in_=outsb[64:128, :],
                        func=mybir.ActivationFunctionType.Tanh,
                    )
                    nc.vector.tensor_tensor(
                        out=tt[64:128, :], in0=sog[64:128, :], in1=tt[64:128, :],
                        op=mybir.AluOpType.mult,
                    )
                    nc.scalar.activation(
                        out=outsb[0:64, :], in_=tt[64:128, :],
                        func=mybir.ActivationFunctionType.Copy,
                    )
                    nc.sync.dma_start(
                        out=outT_d[:, tl * P : (tl + 1) * P], in_=outsb[:]
                    )
    from concourse import mybir as _mb
    _mb.codegen_inst_isa_subclasses(nc)
    _split_multi_waits(nc, mybir)
    return nc


def kernel(feat, src0, dst0, src1, dst1, W_ih, W_hh, b_ih, b_hh):
    global LAST_EXEC_NS
    feat = np.asarray(feat, np.float32)
    src0 = np.asarray(src0, np.int64); dst0 = np.asarray(dst0, np.int64)
    src1 = np.asarray(src1, np.int64); dst1 = np.asarray(dst1, np.int64)
    per_core, shared, node_of_slot, meta = _host_prep(
        feat, src0, dst0, src1, dst1,
        np.asarray(W_ih, np.float32), np.asarray(W_hh, np.float32),
        np.asarray(b_ih, np.float32), np.asarray(b_hh, np.float32),
    )
    nc = _build_nc(meta)
    in_maps = [{**shared, **pc} for pc in per_core]
    from concourse.bass_utils import run_bass_kernel_spmd
    if TRACE:
        import axon_prof  # noqa
    res = run_bass_kernel_spmd(nc, in_maps, list(range(NCORES)), trace=TRACE)
    LAST_EXEC_NS = res.exec_time_ns
    out = np.zeros((N_NODES, H), np.float32)
    for c in range(NCORES):
        oT = res.results[c]["outT"]          # [128, SLOTS]
        valid = node_of_slot[c] >= 0
        nodes = node_of_slot[c][valid]
        blk = oT.T[valid]                    # [n, 128]: cols 0:64=h1, 64:128=c1
        out[nodes] = blk
    return out
